# revision 1
# baseline (speedup 1.0000x reference)
"""TRN2 Bass kernel for a GPT transformer block (B=4, T=2048, C=1024, H=16, dff=4096).

Sharding: 8 NeuronCores, core c = (batch b=c//2, parity p=c%2). Each core owns
the interleaved 128-row sequence chunks {2j+p : j<8} of batch b (this balances
causal-attention work between the two cores of a batch), computes full-sequence
k/v for its batch itself (no collectives), and produces its 1024 own rows of the
output. All 8 cores run ONE identical SPMD program; per-core behavior differs
only through data: the host permutes each core's x so its own chunks come first
(own-prefix order) and supplies per-parity causal masks.

On-chip layout: activations are feature-major ("transposed", [feature, row]) so
every GEMM contracts along the partition dim and outputs stay feature-major.
Attention computes scores transposed [ki, qi]; softmax needs no max-subtraction
(|scores| is small for this distribution); the denominator comes free from a
ones-column augmented onto V; causality = multiply exp tiles by {0,1} masks.

Precision: float32r (fp32 with 11-bit mantissa, full PE rate) for all GEMMs
except fc2 (bf16; the gelu output is cast there anyway). Weights are pre-rounded
to the f32r grid on the host, pre-arranged into partition-major contiguous slabs
(SWDGE descriptor count scales with discontiguity), and DMA'd directly.
"""
import numpy as np
import ml_dtypes

import concourse.bacc as bacc
import concourse.mybir as mybir
import concourse.tile as tile
from concourse.bass_utils import run_bass_kernel_spmd
from concourse.masks import make_identity

F32 = mybir.dt.float32
F32R = mybir.dt.float32r
BF16 = mybir.dt.bfloat16
AF = mybir.ActivationFunctionType
ALU = mybir.AluOpType
AX = mybir.AxisListType

B, T, C, H, HD, DFF = 4, 2048, 1024, 16, 64, 4096
NCH = T // 128          # 16 sequence chunks of 128
NOWN = 8                # own row chunks per core
R = NOWN * 128          # 1024 own rows
EPS = 1e-5


def _f32r_round(x):
    b = np.ascontiguousarray(x, dtype=np.float32).view(np.uint32).astype(np.uint64)
    b = ((b + 0x800) & 0xFFFFF000).astype(np.uint32)
    return b.view(np.float32)


def _slab(w, n_in_ch, slab_cols):
    """[Cin, Cout] -> [n_slabs, 128, n_in_ch, slab_cols] contiguous slabs."""
    cin, cout = w.shape
    assert cin == n_in_ch * 128 and cout % slab_cols == 0
    b = w.reshape(n_in_ch, 128, cout // slab_cols, slab_cols)
    return np.ascontiguousarray(b.transpose(2, 1, 0, 3))


def _layernorm_tiles(nc, statpool, x_ap, out_ap, tag, eps_ap):
    """LN stats (DVE bn_stats) + apply (ACT) for one [128, C] row tile."""
    bns = statpool.tile([128, 2, 6], F32, tag=f"{tag}bns")
    nc.vector.bn_stats(bns[:, 0, :], x_ap[:, 0:512])
    nc.vector.bn_stats(bns[:, 1, :], x_ap[:, 512:1024])
    ag = statpool.tile([128, 2], F32, tag=f"{tag}ag")
    nc.vector.bn_aggr(ag[:], bns[:])
    sig = statpool.tile([128, 1], F32, tag=f"{tag}sig")
    nc.scalar.activation(sig[:], ag[:, 1:2], AF.Sqrt, bias=eps_ap)
    rsig = statpool.tile([128, 1], F32, tag=f"{tag}rsig")
    nc.vector.reciprocal(rsig[:], sig[:])
    nmr = statpool.tile([128, 1], F32, tag=f"{tag}nmr")
    nc.vector.scalar_tensor_tensor(nmr[:], ag[:, 0:1], -1.0, rsig[:],
                                   op0=ALU.mult, op1=ALU.mult)
    nc.scalar.activation(out_ap, x_ap, AF.Identity, bias=nmr[:], scale=rsig[:])


def build_program(debug=False):
    nc = bacc.Bacc(None, target_bir_lowering=False, enable_partition_id=False)

    x_in = nc.declare_dram_parameter("x", [T, C], F32, isOutput=False)
    wqk_in = nc.declare_dram_parameter("wqk", [8, 128, 8, 256], F32R, isOutput=False)
    bqk_in = nc.declare_dram_parameter("bqk", [2 * C], F32, isOutput=False)
    wv_in = nc.declare_dram_parameter("wv", [128, 8, C], F32R, isOutput=False)
    bv_in = nc.declare_dram_parameter("bv", [C], F32R, isOutput=False)
    wproj_in = nc.declare_dram_parameter("wproj", [4, 128, 8, 256], F32R, isOutput=False)
    bproj_in = nc.declare_dram_parameter("bproj", [C], F32, isOutput=False)
    wfc_in = nc.declare_dram_parameter("wfc", [16, 128, 8, 256], F32R, isOutput=False)
    bfc_in = nc.declare_dram_parameter("bfc", [DFF], F32, isOutput=False)
    wfc2_in = nc.declare_dram_parameter("wfc2", [8, 128, 32, 128], F32R, isOutput=False)
    bfc2_in = nc.declare_dram_parameter("bfc2", [C], F32, isOutput=False)
    masks_in = nc.declare_dram_parameter("masks", [128, 2, 8, 512], F32R, isOutput=False)
    out_d = nc.declare_dram_parameter("out", [R, C], F32, isOutput=True)

    dbg = {}
    if debug:
        for nm, shp, dt_ in [("dbg_hT", [128, 8, T], F32R), ("dbg_qT", [128, 8, R], F32R),
                             ("dbg_yT", [128, 8, R], F32R),
                             ("dbg_h2T", [128, 8, R], F32R), ("dbg_gT", [128, 32, R], F32R)]:
            dbg[nm] = nc.declare_dram_parameter(nm, shp, dt_, isOutput=True)

    kT_d = nc.dram_tensor("kT_scratch", [8, 128, T], F32R)
    x1_d = nc.dram_tensor("x1_scratch", [8, 128, C], F32)
    vn_d = nc.dram_tensor("vn_scratch", [NCH, 128, C], F32R)

    x_r = x_in[:].rearrange("(t p) c -> t p c", p=128)

    with tile.TileContext(nc) as tc:
        with (
            tc.tile_pool(name="persist", bufs=1) as persist,
            tc.tile_pool(name="biasp", bufs=1) as biasp,
        ):
            identity = persist.tile([128, 128], F32)
            make_identity(nc, identity[:])
            ones_f = persist.tile([128, 128], F32)
            nc.gpsimd.memset(ones_f[:], 1.0)
            ones_r = persist.tile([1, 128], F32R)
            nc.scalar.copy(ones_r[:], ones_f[0:1, :])
            eps_t = persist.tile([128, 1], F32)
            nc.gpsimd.memset(eps_t[:], EPS)
            bqk_sb = biasp.tile([128, 16], F32)
            nc.gpsimd.dma_start(out=bqk_sb[:], in_=bqk_in[:].rearrange("(m p) -> p m", p=128))
            bv_sb = biasp.tile([1, C], F32R)
            nc.gpsimd.dma_start(out=bv_sb[:], in_=bv_in[:].rearrange("(o c) -> o c", o=1))
            bproj_sb = biasp.tile([128, 8], F32)
            nc.gpsimd.dma_start(out=bproj_sb[:], in_=bproj_in[:].rearrange("(m p) -> p m", p=128))
            bfc_sb = biasp.tile([128, 32], F32)
            nc.gpsimd.dma_start(out=bfc_sb[:], in_=bfc_in[:].rearrange("(m p) -> p m", p=128))
            bfc2_sb = biasp.tile([128, 8], F32)
            nc.gpsimd.dma_start(out=bfc2_sb[:], in_=bfc2_in[:].rearrange("(m p) -> p m", p=128))

            wv_pool = tc.alloc_tile_pool(name="wv_pool", bufs=1)
            wv_sb = wv_pool.tile([128, 8, C], F32R)
            nc.sync.dma_start(out=wv_sb[:], in_=wv_in[:])

            # ---- Stage 1: LN1 over all T (permuted) rows -> hT [C, T] f32r
            hT_pool = tc.alloc_tile_pool(name="hT_pool", bufs=1)
            hT = hT_pool.tile([128, 8, T], F32R)
            with (
                tc.tile_pool(name="s1w", bufs=3) as s1w,
                tc.tile_pool(name="s1s", bufs=3) as s1s,
                tc.tile_pool(name="s1p", bufs=4, space="PSUM") as s1p,
            ):
                for rt2 in range(NCH // 2):
                    xt2 = s1w.tile([128, 2, C], F32, tag="xt")
                    nc.scalar.dma_start(
                        out=xt2[:], in_=x_r[2 * rt2:2 * rt2 + 2].rearrange("t p c -> p t c"))
                    for sub in range(2):
                        rt = 2 * rt2 + sub
                        ht = s1w.tile([128, C], F32, tag="ht")
                        _layernorm_tiles(nc, s1s, xt2[:, sub, :], ht[:], "s1", eps_t[:])
                        for ci in range(8):
                            pt = s1p.tile([128, 128], F32, tag="pt")
                            nc.tensor.transpose(pt[:], ht[:, ci * 128:(ci + 1) * 128],
                                                identity[:])
                            nc.vector.tensor_copy(hT[:, ci, rt * 128:(rt + 1) * 128], pt[:])

            # ---- Stage 2: qkv GEMMs (hT still alive)
            qT_pool = tc.alloc_tile_pool(name="qT_pool", bufs=1, side="right")
            qT = qT_pool.tile([128, 8, R], F32R)

            if True:
                with (
                    tc.tile_pool(name="s2w", bufs=3) as s2w,
                    tc.tile_pool(name="s2ev", bufs=2) as s2ev,
                    tc.tile_pool(name="s2p", bufs=4, space="PSUM") as s2p,
                ):
                    for rt in range(NCH):
                        vb = s2ev.tile([128, C], F32R, tag="vb")
                        for n in range(2):
                            acc = s2p.tile([128, 512], F32, tag="vacc")
                            for ci in range(8):
                                nc.tensor.matmul(acc[:], hT[:, ci, rt * 128:(rt + 1) * 128],
                                                 wv_sb[:, ci, n * 512:(n + 1) * 512],
                                                 start=(ci == 0), stop=False)
                            nc.tensor.matmul(acc[:], ones_r[:, :],
                                             bv_sb[:, n * 512:(n + 1) * 512],
                                             start=False, stop=True)
                            nc.scalar.activation(vb[:, n * 512:(n + 1) * 512], acc[:],
                                                 AF.Identity)
                        nc.scalar.dma_start(out=vn_d[rt], in_=vb[:])

                    for s in (0, 4, 1, 5, 2, 6, 3, 7):
                        wsl = s2w.tile([128, 8, 256], F32R, tag="wqk")
                        nc.sync.dma_start(out=wsl[:], in_=wqk_in[s])
                        for sub in range(2):
                            m = 2 * s + sub
                            if m < 8:
                                for n in range(2):
                                    acc = s2p.tile([128, 512], F32, tag="qkacc")
                                    for ci in range(8):
                                        nc.tensor.matmul(
                                            acc[:], wsl[:, ci, sub * 128:(sub + 1) * 128],
                                            hT[:, ci, n * 512:(n + 1) * 512],
                                            start=(ci == 0), stop=(ci == 7))
                                    nc.scalar.activation(
                                        qT[:, m, n * 512:(n + 1) * 512], acc[:],
                                        AF.Identity, bias=bqk_sb[:, m:m + 1])
                            else:
                                ktb = s2ev.tile([128, T], F32R, tag="ktb")
                                for n in range(4):
                                    acc = s2p.tile([128, 512], F32, tag="qkacc")
                                    for ci in range(8):
                                        nc.tensor.matmul(
                                            acc[:], wsl[:, ci, sub * 128:(sub + 1) * 128],
                                            hT[:, ci, n * 512:(n + 1) * 512],
                                            start=(ci == 0), stop=(ci == 7))
                                    nc.scalar.activation(
                                        ktb[:, n * 512:(n + 1) * 512], acc[:],
                                        AF.Identity, bias=bqk_sb[:, m:m + 1])
                                nc.scalar.dma_start(out=kT_d[m - 8], in_=ktb[:])
            if debug:
                nc.gpsimd.dma_start(out=dbg["dbg_hT"][:], in_=hT[:])
                nc.gpsimd.dma_start(out=dbg["dbg_qT"][:], in_=qT[:])
            hT_pool.release()
            wv_pool.release()

            # ---- Stage 3: attention (scoresT layout, V augmented with ones col)
            mask_pool = tc.alloc_tile_pool(name="mask_pool", bufs=1, side="right")
            m_sb = mask_pool.tile([128, 2, 8, 512], F32R)
            nc.sync.dma_start(out=m_sb[:], in_=masks_in[:])
            yT_pool = tc.alloc_tile_pool(name="yT_pool", bufs=1)
            yT = yT_pool.tile([128, 8, R], F32R)

            with (
                tc.tile_pool(name="kv_pool", bufs=2) as kv_pool,
                tc.tile_pool(name="s3w", bufs=6) as s3w,
                tc.tile_pool(name="s3p", bufs=4, space="PSUM") as s3p,
                tc.tile_pool(name="s3yp", bufs=2, space="PSUM") as s3yp,
                tc.tile_pool(name="s3rp", bufs=2, space="PSUM") as s3rp,
            ):
                for hp in range(8):
                    kt = kv_pool.tile([128, T], F32R, tag="kt")
                    nc.sync.dma_start(out=kt[:], in_=kT_d[hp])
                    vn = kv_pool.tile([128, NCH, 2, HD + 1], F32R, tag="vn")
                    for sub_ in range(2):
                        off = hp * 128 + sub_ * HD
                        nc.sync.dma_start(
                            out=vn[:, :, sub_, 0:HD],
                            in_=vn_d[:, :, off:off + HD].rearrange("t p d -> p t d"))
                    with nc.allow_low_precision(reason="f32r ones fill"):
                        nc.scalar.activation(
                            vn[:, :, :, HD],
                            ones_f[:, :32].rearrange("p (t s) -> p t s", s=2), AF.Identity)
                    for sub in range(2):
                        ph = 64 * sub
                        for g in range(2):
                            qsl = qT[ph:ph + 64, hp, g * 512:(g + 1) * 512]
                            ya = s3yp.tile([65, 512], F32, tag="ya")
                            kcs = list(range(0, 4 * (g + 1))) + \
                                list(range(8, 8 + 4 * (g + 1)))
                            for idx, kc in enumerate(kcs):
                                if 4 * g <= kc < 4 * g + 4:
                                    mi = kc - 4 * g
                                elif 8 + 4 * g <= kc:
                                    mi = 4 + (kc - 8 - 4 * g)
                                else:
                                    mi = None
                                # causally-valid qi range (par=1 bound; masks
                                # zero the rest for par=0); keep >=256 for f32r
                                off = 0 if mi is None else min(mi % 4, 2) * 128
                                w = 512 - off
                                sc = s3p.tile([128, 512], F32, tag="sc")
                                nc.tensor.matmul(
                                    sc[:, :w], kt[ph:ph + 64, kc * 128:(kc + 1) * 128],
                                    qsl[:, off:], start=True, stop=True)
                                et = s3w.tile([128, 512], F32R, tag="et")
                                nc.scalar.activation(et[:, :w], sc[:, :w], AF.Exp)
                                if mi is not None:
                                    nc.vector.tensor_tensor(et[:, :w], et[:, :w],
                                                            m_sb[:, g, mi, off:],
                                                            ALU.mult)
                                nc.tensor.matmul(ya[:, off:], vn[:, kc, sub, :],
                                                 et[:, :w],
                                                 start=(idx == 0),
                                                 stop=(idx == len(kcs) - 1))
                            rec = s3w.tile([1, 512], F32R, tag="rec")
                            with nc.allow_low_precision(reason="f32r softmax recip"):
                                nc.vector.reciprocal(rec[:], ya[64:65, :])
                            rb = s3rp.tile([64, 512], F32, tag="rb")
                            nc.tensor.matmul(rb[:], ones_r[:, :64], rec[:],
                                             start=True, stop=True)
                            yf = s3w.tile([64, 512], F32, tag="yf")
                            nc.vector.tensor_copy(yf[:], ya[:64, :])
                            ytmp = s3w.tile([64, 512], F32R, tag="ytmp")
                            with nc.allow_low_precision(reason="f32r attn out"):
                                nc.vector.tensor_tensor(ytmp[:], yf[:], rb[:], ALU.mult)
                            nc.gpsimd.dma_start(
                                out=yT[ph:ph + 64, hp, g * 512:(g + 1) * 512],
                                in_=ytmp[:])

            if debug:
                nc.gpsimd.dma_start(out=dbg["dbg_yT"][:], in_=yT[:])
            mask_pool.release()
            qT_pool.release()

            # ---- Stage 4: proj + residual -> x1 (DRAM); LN2 -> h2T
            h2T_pool = tc.alloc_tile_pool(name="h2T_pool", bufs=1, side="right")
            h2T = h2T_pool.tile([128, 8, R], F32R)
            with (
                tc.tile_pool(name="s4wp", bufs=1) as s4wp,
                tc.tile_pool(name="s4w", bufs=2) as s4w,
                tc.tile_pool(name="s4pj", bufs=2) as s4pj,
                tc.tile_pool(name="s4s", bufs=3) as s4s,
                tc.tile_pool(name="s4p", bufs=3, space="PSUM") as s4p,
                tc.tile_pool(name="s4tp", bufs=2, space="PSUM") as s4tp,
            ):
                pjw = []
                for s in range(4):
                    w4 = s4wp.tile([128, 8, 256], F32R, tag=f"wpj{s}")
                    nc.sync.dma_start(out=w4[:], in_=wproj_in[s])
                    pjw.append(w4)
                for n in range(2):
                    pjt = []
                    for s in range(4):
                        for sub in range(2):
                            m = 2 * s + sub
                            acc = s4p.tile([128, 512], F32, tag="pjacc")
                            for ci in range(8):
                                nc.tensor.matmul(acc[:],
                                                 pjw[s][:, ci, sub * 128:(sub + 1) * 128],
                                                 yT[:, ci, n * 512:(n + 1) * 512],
                                                 start=(ci == 0), stop=(ci == 7))
                            ev = s4pj.tile([128, 512], F32, tag=f"pjev{m}")
                            nc.scalar.activation(ev[:], acc[:], AF.Identity,
                                                 bias=bproj_sb[:, m:m + 1])
                            pjt.append(ev)
                    for jj in range(4):
                        j = n * 4 + jj
                        xo = s4w.tile([128, C], F32, tag="xo")
                        nc.sync.dma_start(out=xo[:], in_=x_r[j])
                        x1t = s4w.tile([128, C], F32, tag="x1t")
                        for m in range(8):
                            pt = s4tp.tile([128, 128], F32, tag="pjt")
                            nc.tensor.transpose(
                                pt[:], pjt[m][:, jj * 128:(jj + 1) * 128], identity[:])
                            nc.vector.tensor_tensor(
                                x1t[:, m * 128:(m + 1) * 128],
                                pt[:], xo[:, m * 128:(m + 1) * 128], ALU.add)
                        nc.scalar.dma_start(out=x1_d[j], in_=x1t[:])
                        h2 = s4w.tile([128, C], F32, tag="h2")
                        _layernorm_tiles(nc, s4s, x1t[:], h2[:], "s4", eps_t[:])
                        for ci in range(8):
                            pt = s4tp.tile([128, 128], F32, tag="h2t")
                            nc.tensor.transpose(pt[:], h2[:, ci * 128:(ci + 1) * 128],
                                                identity[:])
                            nc.vector.tensor_copy(h2T[:, ci, j * 128:(j + 1) * 128], pt[:])

            if debug:
                nc.gpsimd.dma_start(out=dbg["dbg_h2T"][:], in_=h2T[:])
            yT_pool.release()

            # ---- Stage 5: MLP fc1 -> gelu -> gT (bf16)
            gT_pool = tc.alloc_tile_pool(name="gT_pool", bufs=1)
            gT = gT_pool.tile([128, 32, R], F32R)
            if True:
                with (
                    tc.tile_pool(name="s5w", bufs=3) as s5w,
                    tc.tile_pool(name="s5p", bufs=4, space="PSUM") as s5p,
                ):
                    for s in range(16):
                        wsl = s5w.tile([128, 8, 256], F32R, tag="wfc")
                        nc.gpsimd.dma_start(out=wsl[:], in_=wfc_in[s])
                        for sub in range(2):
                            m = 2 * s + sub
                            for n in range(2):
                                acc = s5p.tile([128, 512], F32, tag="facc")
                                for ci in range(8):
                                    nc.tensor.matmul(
                                        acc[:], wsl[:, ci, sub * 128:(sub + 1) * 128],
                                        h2T[:, ci, n * 512:(n + 1) * 512],
                                        start=(ci == 0), stop=(ci == 7))
                                nc.scalar.activation(
                                    gT[:, m, n * 512:(n + 1) * 512], acc[:],
                                    AF.Gelu_apprx_tanh, bias=bfc_sb[:, m:m + 1])
            if debug:
                nc.gpsimd.dma_start(out=dbg["dbg_gT"][:], in_=gT[:])
            h2T_pool.release()
            # ---- Stage 6: fc2 + residual -> out
            with (
                tc.tile_pool(name="s6w", bufs=2) as s6w,
                tc.tile_pool(name="s6ev", bufs=1) as s6ev,
                tc.tile_pool(name="s6p", bufs=2, space="PSUM") as s6p,
                tc.tile_pool(name="s6tp", bufs=2, space="PSUM") as s6tp,
            ):
                for half in range(2):
                    mev = []
                    for m2 in range(4):
                        m = half * 4 + m2
                        wsl = s6w.tile([128, 32, 128], F32R, tag="wfc2")
                        nc.scalar.dma_start(out=wsl[:], in_=wfc2_in[m])
                        acc = s6p.tile([128, 1024], F32, tag="macc")
                        for n in range(2):
                            for df in range(32):
                                nc.tensor.matmul(
                                    acc[:, n * 512:(n + 1) * 512],
                                    wsl[:, df, :],
                                    gT[:, df, n * 512:(n + 1) * 512],
                                    start=(df == 0), stop=(df == 31))
                        ev = s6ev.tile([128, 1024], F32, tag=f"mev{m2}")
                        nc.scalar.activation(ev[:], acc[:], AF.Identity,
                                             bias=bfc2_sb[:, m:m + 1])
                        mev.append(ev)
                    for j in range(8):
                        x1j = s6w.tile([128, 512], F32, tag="x1j")
                        nc.sync.dma_start(
                            out=x1j[:], in_=x1_d[j][:, half * 512:(half + 1) * 512])
                        stg = s6w.tile([128, 512], F32, tag="stg")
                        for m2 in range(4):
                            oc = half * 4 + m2
                            pt = s6tp.tile([128, 128], F32, tag="mt")
                            nc.tensor.transpose(
                                pt[:], mev[m2][:, j * 128:(j + 1) * 128], identity[:])
                            nc.vector.tensor_tensor(
                                stg[:, m2 * 128:(m2 + 1) * 128], pt[:],
                                x1j[:, m2 * 128:(m2 + 1) * 128], ALU.add)
                        nc.scalar.dma_start(
                            out=out_d[j * 128:(j + 1) * 128,
                                      half * 512:(half + 1) * 512],
                            in_=stg[:])
            gT_pool.release()

    nc.compile()
    return nc


_NC = None


def _host_prepare(x, ln1_w, ln1_b, w_attn, b_attn, w_proj, b_proj,
                  ln2_w, ln2_b, w_fc, b_fc, w_fc2, b_fc2):
    f32 = np.float32
    ln1_w = np.asarray(ln1_w, f32); ln1_b = np.asarray(ln1_b, f32)
    w_attn = np.asarray(w_attn, f32); b_attn = np.asarray(b_attn, f32)
    scale = np.zeros((3 * C,), f32)
    scale[:C] = 0.125
    scale[C:] = 1.0
    w_full = ln1_w[:, None] * w_attn * scale[None, :]
    b_full = (ln1_b @ w_attn + b_attn) * scale
    wv_pm = _f32r_round(w_full[:, 2 * C:]).reshape(8, 128, C).transpose(1, 0, 2)
    shared = {
        "wqk": _slab(_f32r_round(w_full[:, :2 * C]), 8, 256),
        "bqk": np.ascontiguousarray(b_full[:2 * C]),
        "wv": np.ascontiguousarray(wv_pm),
        "bv": _f32r_round(b_full[2 * C:]),
        "wproj": _slab(_f32r_round(np.asarray(w_proj, f32)), 8, 256),
        "bproj": np.asarray(b_proj, f32),
        "wfc": _slab(_f32r_round(np.asarray(ln2_w, f32)[:, None] * np.asarray(w_fc, f32)),
                     8, 256),
        "bfc": np.asarray(ln2_b, f32) @ np.asarray(w_fc, f32) + np.asarray(b_fc, f32),
        "wfc2": _slab(_f32r_round(np.asarray(w_fc2, f32)), 32, 128),
        "bfc2": np.asarray(b_fc2, f32),
    }
    # masks[p]: [128 (ki within chunk), 2 (qgroup), 8 (mask slot), 512 (qi)]
    masks = []
    ki = np.arange(128)
    for p in range(2):
        mk = np.zeros((128, 2, 8, 512), f32)
        for g in range(2):
            qc_i = np.arange(512) // 128          # i within group
            qr = np.arange(512) % 128
            q_real = (2 * (4 * g + qc_i) + p) * 128 + qr    # [512]
            for slot in range(8):
                if slot < 4:
                    real_chunk = 2 * (4 * g + slot) + p      # own keys
                else:
                    real_chunk = 2 * (4 * g + slot - 4) + 1 - p  # other-parity keys
                k_real = real_chunk * 128 + ki               # [128]
                mk[:, g, slot, :] = (k_real[:, None] <= q_real[None, :])
        masks.append(mk)
    return shared, masks


def kernel(**inputs):
    global _NC
    if _NC is None:
        _NC = build_program()
    nc = _NC
    x = np.asarray(inputs["x"], np.float32)
    shared, masks = _host_prepare(**inputs)
    in_maps = []
    for c in range(8):
        b, p = c // 2, c % 2
        perm = [2 * j + p for j in range(8)] + [2 * j + 1 - p for j in range(8)]
        xp = np.ascontiguousarray(
            x[b].reshape(NCH, 128, C)[perm].reshape(T, C))
        im = dict(shared)
        im["x"] = xp
        im["masks"] = masks[p]
        in_maps.append(im)
    res = run_bass_kernel_spmd(nc, in_maps, list(range(8)), trace=False).results
    out = np.empty((B, T, C), np.float32)
    for c in range(8):
        b, p = c // 2, c % 2
        oc = res[c]["out"].reshape(NOWN, 128, C)
        for j in range(NOWN):
            out[b, (2 * j + p) * 128:(2 * j + p + 1) * 128, :] = oc[j]
    return out



# revision 30
# speedup vs baseline: 1.3642x; 1.3642x over previous
"""TRN2 Bass kernel for a GPT transformer block (B=4, T=2048, C=1024, H=16, dff=4096).

Sharding: 8 NeuronCores, core c = (batch b=c//2, parity p=c%2). Each core owns
the interleaved 128-row sequence chunks {2j+p : j<8} of batch b (this balances
causal-attention work between the two cores of a batch), computes full-sequence
k/v for its batch itself (no collectives), and produces its 1024 own rows of the
output. All 8 cores run ONE identical SPMD program; per-core behavior differs
only through data: the host permutes each core's x so its own chunks come first
(own-prefix order) and supplies per-parity causal masks.

On-chip layout: activations are feature-major ("transposed", [feature, row]) so
every GEMM contracts along the partition dim and outputs stay feature-major.
Attention computes scores transposed [ki, qi]; softmax needs no max-subtraction
(|scores| is small for this distribution); the denominator comes free from a
ones-column augmented onto V; causality = multiply exp tiles by {0,1} masks.

Precision: the large GEMMs (qkv, proj, fc1) run in fp8e4m3 with the DoubleRow
perf mode (contracts 256 rows/matmul at 0.5 cyc/row = 4x f32r throughput).
fc1 weights are hi+lo fp8 pairs sharing one scale (W ~ W8hi + W8lo gives
~13-bit weights) and the LN2 activations get the same hi+lo split, so fc1 is
a 3-term compensated product. fc2 runs bf16 x bf16 (1 cyc/row). The
attention inner loops (scores/softmax/A@V) stay float32r as before.
"""
import numpy as np
import ml_dtypes

import concourse.bacc as bacc
import concourse.mybir as mybir
import concourse.tile as tile
from concourse.bass_utils import run_bass_kernel_spmd
from concourse.masks import make_identity

F32 = mybir.dt.float32
F32R = mybir.dt.float32r
BF16 = mybir.dt.bfloat16
F8 = mybir.dt.float8e4
AF = mybir.ActivationFunctionType
ALU = mybir.AluOpType
DRm = mybir.MatmulPerfMode.DoubleRow
E4 = ml_dtypes.float8_e4m3
BF = ml_dtypes.bfloat16

B, T, C, H, HD, DFF = 4, 2048, 1024, 16, 64, 4096
NCH = T // 128          # 16 sequence chunks of 128
NOWN = 8                # own row chunks per core
R = NOWN * 128          # 1024 own rows
EPS = 1e-5

SH = 32.0               # fp8 scale for LN1 output h
SH2 = 16.0              # fp8 scale for LN2 output h2
SY = 32.0               # fp8 scale for attention output y


def _f32r_round(x):
    b = np.ascontiguousarray(x, dtype=np.float32).view(np.uint32).astype(np.uint64)
    b = ((b + 0x800) & 0xFFFFF000).astype(np.uint32)
    return b.view(np.float32)


def _pow2_scale(a, tgt=160.0):
    return float(2.0 ** np.floor(np.log2(tgt / max(float(np.abs(a).max()), 1e-30))))


def _q8(w, s):
    return np.clip(np.asarray(w, np.float32) * s, -240.0, 240.0).astype(E4)


def _slab8(w, s, ncol):
    """[Cin, Cout] f32 -> fp8 slabs [nslab, 128, Cin/128, ncol] (chunk-major free dim)."""
    cin, cout = w.shape
    q = _q8(w, s).reshape(cin // 128, 128, cout // ncol, ncol)
    return np.ascontiguousarray(q.transpose(2, 1, 0, 3))


def _layernorm_tiles(nc, statpool, x_ap, out_ap, tag, eps_ap):
    """LN stats (DVE bn_stats) + apply (ACT) for one [128, C] row tile."""
    bns = statpool.tile([128, 2, 6], F32, tag=f"{tag}bns")
    nc.vector.bn_stats(bns[:, 0, :], x_ap[:, 0:512])
    nc.vector.bn_stats(bns[:, 1, :], x_ap[:, 512:1024])
    ag = statpool.tile([128, 2], F32, tag=f"{tag}ag")
    nc.vector.bn_aggr(ag[:], bns[:])
    sig = statpool.tile([128, 1], F32, tag=f"{tag}sig")
    nc.scalar.activation(sig[:], ag[:, 1:2], AF.Sqrt, bias=eps_ap)
    rsig = statpool.tile([128, 1], F32, tag=f"{tag}rsig")
    nc.vector.reciprocal(rsig[:], sig[:])
    nmr = statpool.tile([128, 1], F32, tag=f"{tag}nmr")
    nc.vector.scalar_tensor_tensor(nmr[:], ag[:, 0:1], -1.0, rsig[:],
                                   op0=ALU.mult, op1=ALU.mult)
    nc.scalar.activation(out_ap, x_ap, AF.Identity, bias=nmr[:], scale=rsig[:])


def build_program(sw_scales, debug=False):
    swqk, swv, swp, sw1, sw2 = sw_scales
    nc = bacc.Bacc(None, target_bir_lowering=False, enable_partition_id=False)

    x_in = nc.declare_dram_parameter("x", [T, C], F32, isOutput=False)
    wqk_in = nc.declare_dram_parameter("wqk", [8, 128, 8, 256], F8, isOutput=False)
    bqk_in = nc.declare_dram_parameter("bqk", [2 * C], F32, isOutput=False)
    wv_in = nc.declare_dram_parameter("wv", [128, 8, C], F8, isOutput=False)
    bv_in = nc.declare_dram_parameter("bv", [C], F32R, isOutput=False)
    wproj_in = nc.declare_dram_parameter("wproj", [4, 128, 8, 256], F8, isOutput=False)
    bproj_in = nc.declare_dram_parameter("bproj", [C], F32, isOutput=False)
    wfc_in = nc.declare_dram_parameter("wfc", [16, 128, 8, 2, 256], F8, isOutput=False)
    bfc_in = nc.declare_dram_parameter("bfc", [DFF], F32, isOutput=False)
    wfc2_in = nc.declare_dram_parameter("wfc2", [8, 128, 32, 2, 128], F8, isOutput=False)
    bfc2_in = nc.declare_dram_parameter("bfc2", [C], F32, isOutput=False)
    masks_in = nc.declare_dram_parameter("masks", [128, 2, 8, 512], F8, isOutput=False)
    out_d = nc.declare_dram_parameter("out", [R, C], F32, isOutput=True)

    vn_d = nc.dram_tensor("vn_scratch", [NCH, 128, C], F32R)

    x_r = x_in[:].rearrange("(t p) c -> t p c", p=128)

    qe_scale = 1.0 / (SH * swqk)
    ve_scale = 1.0 / (SH * swv)
    pe_scale = 1.0 / (SY * swp)
    ge_scale = 1.0 / (SH2 * sw1)
    f2_scale = 1.0 / sw2

    with tile.TileContext(nc) as tc:
        with (
            tc.tile_pool(name="persist", bufs=1) as persist,
            tc.tile_pool(name="biasp", bufs=1) as biasp,
        ):
            identity = persist.tile([128, 128], F32)
            make_identity(nc, identity[:])
            identb = persist.tile([128, 128], BF16)
            nc.scalar.copy(identb[:], identity[:])
            ones_f = persist.tile([128, 128], F32)
            nc.gpsimd.memset(ones_f[:], 1.0)
            ones_r = persist.tile([1, 128], F32R)
            nc.scalar.copy(ones_r[:], ones_f[0:1, :])
            sy_f = persist.tile([1, 64], F32)
            nc.gpsimd.memset(sy_f[:], SY)
            sy_r = persist.tile([1, 64], F32R)
            nc.scalar.copy(sy_r[:], sy_f[:])
            eps_t = persist.tile([128, 1], F32)
            nc.gpsimd.memset(eps_t[:], EPS)
            bqk_sb = biasp.tile([128, 16], F32)
            nc.scalar.dma_start(out=bqk_sb[:], in_=bqk_in[:].rearrange("(m p) -> p m", p=128))
            bv_sb = biasp.tile([1, C], F32R)
            nc.scalar.dma_start(out=bv_sb[:], in_=bv_in[:].rearrange("(o c) -> o c", o=1))
            bproj_sb = biasp.tile([128, 8], F32)
            nc.scalar.dma_start(out=bproj_sb[:], in_=bproj_in[:].rearrange("(m p) -> p m", p=128))
            bfc_sb = biasp.tile([128, 32], F32)
            nc.scalar.dma_start(out=bfc_sb[:], in_=bfc_in[:].rearrange("(m p) -> p m", p=128))
            bfc2_sb = biasp.tile([128, 8], F32)
            nc.scalar.dma_start(out=bfc2_sb[:], in_=bfc2_in[:].rearrange("(m p) -> p m", p=128))

            wv_pool = tc.alloc_tile_pool(name="wv_pool", bufs=1)
            wv_sb = wv_pool.tile([128, 8, C], F8)
            nc.sync.dma_start(out=wv_sb[:], in_=wv_in[:])

            # ---- Stage 1+2 interleaved: per 512-row group: LN1 -> v -> q/k GEMMs
            qT_pool = tc.alloc_tile_pool(name="qT_pool", bufs=1, side="right")
            qT = qT_pool.tile([128, 8, R], F32R)
            kT_pool = tc.alloc_tile_pool(name="kT_pool", bufs=1, side="right")
            kTs = kT_pool.tile([128, 8, T], F32R)
            wqk_pool = tc.alloc_tile_pool(name="wqk_pool", bufs=1)
            wqk_sb = wqk_pool.tile([128, 8, 8, 256], F8)
            nc.sync.dma_start(out=wqk_sb[:], in_=wqk_in[:].rearrange("s p c m -> p s c m"))
            hT_pool = tc.alloc_tile_pool(name="hT_pool", bufs=1)
            hT = hT_pool.tile([128, 8, T], F8)
            with (
                tc.tile_pool(name="s1w", bufs=3) as s1w,
                tc.tile_pool(name="s1s", bufs=3) as s1s,
                tc.tile_pool(name="s1p", bufs=2, space="PSUM") as s1p,
                tc.tile_pool(name="s2ev", bufs=2) as s2ev,
                tc.tile_pool(name="s2p", bufs=3, space="PSUM") as s2p,
            ):
                for ng in range(4):
                    for rt2 in range(2 * ng, 2 * ng + 2):
                        xt2 = s1w.tile([128, 2, C], F32, tag="xt")
                        nc.gpsimd.dma_start(
                            out=xt2[:], in_=x_r[2 * rt2:2 * rt2 + 2].rearrange("t p c -> p t c"))
                        for sub in range(2):
                            rt = 2 * rt2 + sub
                            ht = s1w.tile([128, C], BF16, tag="ht")
                            _layernorm_tiles(nc, s1s, xt2[:, sub, :], ht[:], "s1", eps_t[:])
                            for ci in range(8):
                                pt = s1p.tile([128, 128], BF16, tag="pt")
                                nc.tensor.transpose(pt[:], ht[:, ci * 128:(ci + 1) * 128],
                                                    identb[:])
                                nc.vector.tensor_scalar_mul(
                                    hT[:, ci, rt * 128:(rt + 1) * 128], pt[:], SH)
                            vb = s2ev.tile([128, C], F32R, tag="vb")
                            for n in range(2):
                                acc = s2p.tile([128, 512], F32, tag="vacc")
                                for c in range(4):
                                    nc.tensor.matmul(
                                        acc[:],
                                        hT[:, 2 * c:2 * c + 2, rt * 128:(rt + 1) * 128],
                                        wv_sb[:, 2 * c:2 * c + 2, n * 512:(n + 1) * 512],
                                        start=(c == 0), stop=False, perf_mode=DRm)
                                nc.tensor.matmul(acc[:], ones_r[:, :],
                                                 bv_sb[:, n * 512:(n + 1) * 512],
                                                 start=False, stop=True)
                                nc.scalar.activation(vb[:, n * 512:(n + 1) * 512], acc[:],
                                                     AF.Identity, scale=ve_scale)
                            nc.scalar.dma_start(out=vn_d[rt], in_=vb[:])
                    # all q/k GEMMs whose moving slice lives in this 512-row group
                    for s in range(8):
                        for sub in range(2):
                            m = 2 * s + sub
                            if m < 8 and ng >= 2:
                                continue
                            acc = s2p.tile([128, 512], F32, tag="qkacc")
                            for c in range(4):
                                nc.tensor.matmul(
                                    acc[:],
                                    wqk_sb[:, s, 2 * c:2 * c + 2, sub * 128:(sub + 1) * 128],
                                    hT[:, 2 * c:2 * c + 2, ng * 512:(ng + 1) * 512],
                                    start=(c == 0), stop=(c == 3), perf_mode=DRm)
                            dst = (qT[:, m, ng * 512:(ng + 1) * 512] if m < 8 else
                                   kTs[:, m - 8, ng * 512:(ng + 1) * 512])
                            nc.vector.tensor_scalar(
                                dst, acc[:], qe_scale, bqk_sb[:, m:m + 1],
                                op0=ALU.mult, op1=ALU.add)
            hT_pool.release()
            wqk_pool.release()
            wv_pool.release()

            # ---- Stage 3: attention (f32r scoresT layout, V augmented with ones col)
            mask_pool = tc.alloc_tile_pool(name="mask_pool", bufs=1, side="right")
            m_sb = mask_pool.tile([128, 2, 8, 512], F8)
            nc.sync.dma_start(out=m_sb[:], in_=masks_in[:])
            yT_pool = tc.alloc_tile_pool(name="yT_pool", bufs=1)
            yT = yT_pool.tile([128, 8, R], F8)

            with (
                tc.tile_pool(name="kv_pool", bufs=2) as kv_pool,
                tc.tile_pool(name="s3w", bufs=9) as s3w,
                tc.tile_pool(name="s3p", bufs=4, space="PSUM") as s3p,
                tc.tile_pool(name="s3yp", bufs=2, space="PSUM") as s3yp,
                tc.tile_pool(name="s3rp", bufs=2, space="PSUM") as s3rp,
            ):
                for hp in range(8):
                    kt = kTs[:, hp, :]
                    vn = kv_pool.tile([128, NCH, 2, HD + 1], F32R, tag="vn")
                    for sub_ in range(2):
                        off = hp * 128 + sub_ * HD
                        nc.sync.dma_start(
                            out=vn[:, :, sub_, 0:HD],
                            in_=vn_d[:, :, off:off + HD].rearrange("t p d -> p t d"))
                    with nc.allow_low_precision(reason="f32r ones fill"):
                        nc.scalar.activation(
                            vn[:, :, :, HD],
                            ones_f[:, :32].rearrange("p (t s) -> p t s", s=2), AF.Identity)
                    for sub in range(2):
                        ph = 64 * sub
                        for g in range(2):
                            qsl = qT[ph:ph + 64, hp, g * 512:(g + 1) * 512]
                            ya = s3yp.tile([65, 512], F32, tag="ya")
                            kcs = list(range(0, 4 * (g + 1))) + \
                                list(range(8, 8 + 4 * (g + 1)))
                            for idx, kc in enumerate(kcs):
                                if 4 * g <= kc < 4 * g + 4:
                                    mi = kc - 4 * g
                                elif 8 + 4 * g <= kc:
                                    mi = 4 + (kc - 8 - 4 * g)
                                else:
                                    mi = None
                                # causally-valid qi range (par=1 bound; masks
                                # zero the rest for par=0); keep >=256 for f32r
                                off = 0 if mi is None else min(mi % 4, 2) * 128
                                w = 512 - off
                                sc = s3p.tile([128, 512], F32, tag="sc")
                                nc.tensor.matmul(
                                    sc[:, :w], kTs[ph:ph + 64, hp, kc * 128:(kc + 1) * 128],
                                    qsl[:, off:], start=True, stop=True)
                                et = s3w.tile([128, 512], F32R, tag="et")
                                nc.scalar.activation(et[:, :w], sc[:, :w], AF.Exp)
                                if mi is not None:
                                    nc.vector.tensor_tensor(et[:, :w], et[:, :w],
                                                            m_sb[:, g, mi, off:],
                                                            ALU.mult)
                                nc.tensor.matmul(ya[:, off:], vn[:, kc, sub, :],
                                                 et[:, :w],
                                                 start=(idx == 0),
                                                 stop=(idx == len(kcs) - 1))
                            rec = s3w.tile([1, 512], F32R, tag="rec")
                            with nc.allow_low_precision(reason="f32r softmax recip"):
                                nc.vector.reciprocal(rec[:], ya[64:65, :])
                            rb = s3rp.tile([64, 512], F32, tag="rb")
                            nc.tensor.matmul(rb[:], sy_r[:, :], rec[:],
                                             start=True, stop=True)
                            yf = s3w.tile([64, 512], F32, tag="yf")
                            nc.vector.tensor_copy(yf[:], ya[:64, :])
                            ytmp = s3w.tile([64, 512], F8, tag="ytmp")
                            with nc.allow_low_precision(reason="fp8 attn out"):
                                nc.vector.tensor_tensor(ytmp[:], yf[:], rb[:], ALU.mult)
                            nc.gpsimd.dma_start(
                                out=yT[ph:ph + 64, hp, g * 512:(g + 1) * 512],
                                in_=ytmp[:])

            mask_pool.release()
            kT_pool.release()
            qT_pool.release()

            # ---- Stage 4: proj (fp8 DR) + residual -> x1 (SBUF); LN2 -> h2T8 + lo
            x1_pool = tc.alloc_tile_pool(name="x1_pool", bufs=1)
            x1s = x1_pool.tile([128, 8, C], F32)
            h2T_pool = tc.alloc_tile_pool(name="h2T_pool", bufs=1, side="right")
            h2T = h2T_pool.tile([128, 8, R], F8)
            h2lo_pool = tc.alloc_tile_pool(name="h2lo_pool", bufs=1, side="right")
            h2loT = h2lo_pool.tile([128, 8, R], F8)
            with (
                tc.tile_pool(name="s4wp", bufs=1) as s4wp,
                tc.tile_pool(name="s4w", bufs=2) as s4w,
                tc.tile_pool(name="s4pj", bufs=2) as s4pj,
                tc.tile_pool(name="s4s", bufs=3) as s4s,
                tc.tile_pool(name="s4p", bufs=3, space="PSUM") as s4p,
                tc.tile_pool(name="s4tp", bufs=2, space="PSUM") as s4tp,
            ):
                pjw = []
                for s in range(4):
                    w4 = s4wp.tile([128, 8, 256], F8, tag=f"wpj{s}")
                    nc.sync.dma_start(out=w4[:], in_=wproj_in[s])
                    pjw.append(w4)
                for n in range(2):
                    pjt = []
                    for s in range(4):
                        for sub in range(2):
                            m = 2 * s + sub
                            acc = s4p.tile([128, 512], F32, tag="pjacc")
                            for c in range(4):
                                nc.tensor.matmul(
                                    acc[:],
                                    pjw[s][:, 2 * c:2 * c + 2, sub * 128:(sub + 1) * 128],
                                    yT[:, 2 * c:2 * c + 2, n * 512:(n + 1) * 512],
                                    start=(c == 0), stop=(c == 3), perf_mode=DRm)
                            ev = s4pj.tile([128, 512], F32, tag=f"pjev{m}")
                            nc.scalar.activation(ev[:], acc[:], AF.Identity,
                                                 bias=bproj_sb[:, m:m + 1],
                                                 scale=pe_scale)
                            pjt.append(ev)
                    for jj in range(4):
                        j = n * 4 + jj
                        xo = s4w.tile([128, C], F32, tag="xo")
                        nc.sync.dma_start(out=xo[:], in_=x_r[j])
                        for m in range(8):
                            pt = s4tp.tile([128, 128], F32, tag="pjt")
                            nc.tensor.transpose(
                                pt[:], pjt[m][:, jj * 128:(jj + 1) * 128], identity[:])
                            nc.vector.tensor_tensor(
                                x1s[:, j, m * 128:(m + 1) * 128],
                                pt[:], xo[:, m * 128:(m + 1) * 128], ALU.add)
                        h2 = s4w.tile([128, C], BF16, tag="h2")
                        _layernorm_tiles(nc, s4s, x1s[:, j, :], h2[:], "s4", eps_t[:])
                        for ci in range(8):
                            pt = s4tp.tile([128, 128], BF16, tag="h2t")
                            nc.tensor.transpose(pt[:], h2[:, ci * 128:(ci + 1) * 128],
                                                identb[:])
                            nc.vector.tensor_scalar_mul(
                                h2T[:, ci, j * 128:(j + 1) * 128], pt[:], SH2)
                            nc.vector.scalar_tensor_tensor(
                                h2loT[:, ci, j * 128:(j + 1) * 128], pt[:], SH2,
                                h2T[:, ci, j * 128:(j + 1) * 128],
                                op0=ALU.mult, op1=ALU.subtract)


            # ---- Stage 5: MLP fc1 (fp8 DR, 3-term compensated) -> gelu -> gT bf16
            gT_pool = tc.alloc_tile_pool(name="gT_pool", bufs=1)
            gT = gT_pool.tile([128, 32, R], F8)
            with (
                tc.tile_pool(name="s5w", bufs=4) as s5w,
                tc.tile_pool(name="s5p", bufs=4, space="PSUM") as s5p,
            ):
                for s in range(16):
                    wsl = s5w.tile([128, 8, 2, 256], F8, tag="wfc")
                    nc.sync.dma_start(out=wsl[:], in_=wfc_in[s])
                    for sub in range(2):
                        m = 2 * s + sub
                        for n in range(2):
                            acc = s5p.tile([128, 512], F32, tag="facc")
                            for c in range(8):
                                nc.tensor.matmul(
                                    acc[:],
                                    wsl[:, c, :, sub * 128:(sub + 1) * 128],
                                    h2T[:, c, n * 512:(n + 1) * 512]
                                    .unsqueeze(1).broadcast_to([128, 2, 512]),
                                    start=(c == 0), stop=False, perf_mode=DRm)
                            for c in range(4):
                                nc.tensor.matmul(
                                    acc[:],
                                    wsl[:, 2 * c:2 * c + 2, 0, sub * 128:(sub + 1) * 128],
                                    h2loT[:, 2 * c:2 * c + 2, n * 512:(n + 1) * 512],
                                    start=False, stop=(c == 3), perf_mode=DRm)
                            nc.scalar.activation(
                                gT[:, m, n * 512:(n + 1) * 512], acc[:],
                                AF.Gelu_apprx_tanh, bias=bfc_sb[:, m:m + 1],
                                scale=ge_scale)
            h2lo_pool.release()
            h2T_pool.release()

            # ---- Stage 6: fc2 (bf16) + residual -> out
            with (
                tc.tile_pool(name="s6w", bufs=3) as s6w,
                tc.tile_pool(name="s6ev", bufs=1) as s6ev,
                tc.tile_pool(name="s6p", bufs=3, space="PSUM") as s6p,
                tc.tile_pool(name="s6tp", bufs=2, space="PSUM") as s6tp,
            ):
                for half in range(2):
                    mev = []
                    for m2 in range(4):
                        m = half * 4 + m2
                        wsl = s6w.tile([128, 32, 2, 128], F8, tag="wfc2")
                        nc.sync.dma_start(out=wsl[:], in_=wfc2_in[m])
                        acc = s6p.tile([128, 1024], F32, tag="macc")
                        for n in range(2):
                            for df in range(32):
                                nc.tensor.matmul(
                                    acc[:, n * 512:(n + 1) * 512],
                                    wsl[:, df, :, :],
                                    gT[:, df, n * 512:(n + 1) * 512]
                                    .unsqueeze(1).broadcast_to([128, 2, 512]),
                                    start=(df == 0), stop=(df == 31), perf_mode=DRm)
                        ev = s6ev.tile([128, 1024], F32, tag=f"mev{m2}")
                        nc.scalar.activation(ev[:], acc[:], AF.Identity,
                                             bias=bfc2_sb[:, m:m + 1],
                                             scale=f2_scale)
                        mev.append(ev)
                    for j in range(8):
                        stg = s6w.tile([128, 512], F32, tag="stg")
                        for m2 in range(4):
                            pt = s6tp.tile([128, 128], F32, tag="mt")
                            nc.tensor.transpose(
                                pt[:], mev[m2][:, j * 128:(j + 1) * 128], identity[:])
                            nc.vector.tensor_tensor(
                                stg[:, m2 * 128:(m2 + 1) * 128], pt[:],
                                x1s[:, j, half * 512 + m2 * 128:
                                    half * 512 + (m2 + 1) * 128], ALU.add)
                        nc.scalar.dma_start(
                            out=out_d[j * 128:(j + 1) * 128,
                                      half * 512:(half + 1) * 512],
                            in_=stg[:])
            gT_pool.release()
            x1_pool.release()
            yT_pool.release()

    nc.compile()
    return nc


_NC = None
_NC_SCALES = None


def _host_prepare(x, ln1_w, ln1_b, w_attn, b_attn, w_proj, b_proj,
                  ln2_w, ln2_b, w_fc, b_fc, w_fc2, b_fc2):
    f32 = np.float32
    ln1_w = np.asarray(ln1_w, f32); ln1_b = np.asarray(ln1_b, f32)
    w_attn = np.asarray(w_attn, f32); b_attn = np.asarray(b_attn, f32)
    scale = np.zeros((3 * C,), f32)
    scale[:C] = 0.125
    scale[C:] = 1.0
    w_full = ln1_w[:, None] * w_attn * scale[None, :]
    b_full = (ln1_b @ w_attn + b_attn) * scale
    wqk = w_full[:, :2 * C]
    swqk = _pow2_scale(wqk)
    wv = w_full[:, 2 * C:]
    swv = _pow2_scale(wv)
    wproj = np.asarray(w_proj, f32)
    swp = _pow2_scale(wproj)
    wfc = np.asarray(ln2_w, f32)[:, None] * np.asarray(w_fc, f32)
    sw1 = _pow2_scale(wfc)
    wfc2 = np.asarray(w_fc2, f32)
    sw2 = _pow2_scale(wfc2)
    w2hi = _slab8(wfc2, sw2, 128)                    # [32, 128, 32, 128]
    w2hi_full = w2hi.astype(f32).transpose(2, 1, 0, 3).reshape(DFF, C)
    w2lo = _slab8(wfc2 - w2hi_full / sw2, sw2, 128)
    wfc2_slab = np.ascontiguousarray(np.stack([w2hi, w2lo], axis=3))
    # fc1 hi+lo fp8 split sharing one scale: slab [16, 128, 8, 2, 256]
    w1hi = _slab8(wfc, sw1, 256)                     # [16, 128, 8, 256]
    w1hi_full = w1hi.astype(f32).transpose(2, 1, 0, 3).reshape(C, DFF)
    w1lo = _slab8(wfc - w1hi_full / sw1, sw1, 256)
    wfc_slab = np.ascontiguousarray(np.stack([w1hi, w1lo], axis=3))
    # wv fp8 [128, 8, C]: [cin_part, cin_chunk, cout]
    wv8 = _q8(wv, swv).reshape(8, 128, C).transpose(1, 0, 2)
    shared = {
        "wqk": _slab8(wqk, swqk, 256),
        "bqk": np.ascontiguousarray(b_full[:2 * C]),
        "wv": np.ascontiguousarray(wv8),
        "bv": _f32r_round(b_full[2 * C:] * SH * swv),
        "wproj": _slab8(wproj, swp, 256),
        "bproj": np.asarray(b_proj, f32),
        "wfc": wfc_slab,
        "bfc": np.asarray(ln2_b, f32) @ np.asarray(w_fc, f32) + np.asarray(b_fc, f32),
        "wfc2": wfc2_slab,
        "bfc2": np.asarray(b_fc2, f32),
    }
    # masks[p]: [128 (ki within chunk), 2 (qgroup), 8 (mask slot), 512 (qi)]
    masks = []
    ki = np.arange(128)
    for p in range(2):
        mk = np.zeros((128, 2, 8, 512), f32)
        for g in range(2):
            qc_i = np.arange(512) // 128          # i within group
            qr = np.arange(512) % 128
            q_real = (2 * (4 * g + qc_i) + p) * 128 + qr    # [512]
            for slot in range(8):
                if slot < 4:
                    real_chunk = 2 * (4 * g + slot) + p      # own keys
                else:
                    real_chunk = 2 * (4 * g + slot - 4) + 1 - p  # other-parity keys
                k_real = real_chunk * 128 + ki               # [128]
                mk[:, g, slot, :] = (k_real[:, None] <= q_real[None, :])
        masks.append(mk.astype(E4))
    return shared, masks, (swqk, swv, swp, sw1, sw2)


def kernel(**inputs):
    global _NC, _NC_SCALES
    x = np.asarray(inputs["x"], np.float32)
    shared, masks, sw_scales = _host_prepare(**inputs)
    if _NC is None or _NC_SCALES != sw_scales:
        _NC = build_program(sw_scales)
        _NC_SCALES = sw_scales
    nc = _NC
    in_maps = []
    for c in range(8):
        b, p = c // 2, c % 2
        perm = [2 * j + p for j in range(8)] + [2 * j + 1 - p for j in range(8)]
        xp = np.ascontiguousarray(
            x[b].reshape(NCH, 128, C)[perm].reshape(T, C))
        im = dict(shared)
        im["x"] = xp
        im["masks"] = masks[p]
        in_maps.append(im)
    res = run_bass_kernel_spmd(nc, in_maps, list(range(8)), trace=False).results
    out = np.empty((B, T, C), np.float32)
    for c in range(8):
        b, p = c // 2, c % 2
        oc = res[c]["out"].reshape(NOWN, 128, C)
        for j in range(NOWN):
            out[b, (2 * j + p) * 128:(2 * j + p + 1) * 128, :] = oc[j]
    return out            hT_pool.release()
            wqk_pool.release()
            wv_pool.release()

            # ---- Stage 3: attention (f32r scoresT layout, V augmented with ones col)
            mask_pool = tc.alloc_tile_pool(name="mask_pool", bufs=1, side="right")
            m_sb = mask_pool.tile([128, 2, 8, 512], F8)
            nc.sync.dma_start(out=m_sb[:], in_=masks_in[:])
            yT_pool = tc.alloc_tile_pool(name="yT_pool", bufs=1)
            yT = yT_pool.tile([128, 8, R], F8)

            with (
                tc.tile_pool(name="kv_pool", bufs=2) as kv_pool,
                tc.tile_pool(name="s3w", bufs=9) as s3w,
                tc.tile_pool(name="s3p", bufs=4, space="PSUM") as s3p,
                tc.tile_pool(name="s3yp", bufs=2, space="PSUM") as s3yp,
                tc.tile_pool(name="s3rp", bufs=2, space="PSUM") as s3rp,
            ):
                for hp in range(8):
                    kt = kTs[:, hp, :]
                    vn = kv_pool.tile([128, NCH, 2, HD + 1], F32R, tag="vn")
                    for sub_ in range(2):
                        off = hp * 128 + sub_ * HD
                        nc.sync.dma_start(
                            out=vn[:, :, sub_, 0:HD],
                            in_=vn_d[:, :, off:off + HD].rearrange("t p d -> p t d"))
                    with nc.allow_low_precision(reason="f32r ones fill"):
                        nc.scalar.activation(
                            vn[:, :, :, HD],
                            ones_f[:, :32].rearrange("p (t s) -> p t s", s=2), AF.Identity)
                    for sub in range(2):
                        ph = 64 * sub
                        for g in range(2):
                            qsl = qT[ph:ph + 64, hp, g * 512:(g + 1) * 512]
                            ya = s3yp.tile([65, 512], F32, tag="ya")
                            kcs = list(range(0, 4 * (g + 1))) + \
                                list(range(8, 8 + 4 * (g + 1)))
                            for idx, kc in enumerate(kcs):
                                if 4 * g <= kc < 4 * g + 4:
                                    mi = kc - 4 * g
                                elif 8 + 4 * g <= kc:
                                    mi = 4 + (kc - 8 - 4 * g)
                                else:
                                    mi = None
                                # causally-valid qi range (par=1 bound; masks
                                # zero the rest for par=0); keep >=256 for f32r
                                off = 0 if mi is None else min(mi % 4, 2) * 128
                                w = 512 - off
                                sc = s3p.tile([128, 512], F32, tag="sc")
                                nc.tensor.matmul(
                                    sc[:, :w], kTs[ph:ph + 64, hp, kc * 128:(kc + 1) * 128],
                                    qsl[:, off:], start=True, stop=True)
                                et = s3w.tile([128, 512], F32R, tag="et")
                                nc.scalar.activation(et[:, :w], sc[:, :w], AF.Exp)
                                if mi is not None:
                                    nc.vector.tensor_tensor(et[:, :w], et[:, :w],
                                                            m_sb[:, g, mi, off:],
                                                            ALU.mult)
                                nc.tensor.matmul(ya[:, off:], vn[:, kc, sub, :],
                                                 et[:, :w],
                                                 start=(idx == 0),
                                                 stop=(idx == len(kcs) - 1))
                            rec = s3w.tile([1, 512], F32R, tag="rec")
                            with nc.allow_low_precision(reason="f32r softmax recip"):
                                nc.vector.reciprocal(rec[:], ya[64:65, :])
                            rb = s3rp.tile([64, 512], F32, tag="rb")
                            nc.tensor.matmul(rb[:], sy_r[:, :], rec[:],
                                             start=True, stop=True)
                            yf = s3w.tile([64, 512], F32, tag="yf")
                            nc.vector.tensor_copy(yf[:], ya[:64, :])
                            ytmp = s3w.tile([64, 512], F8, tag="ytmp")
                            with nc.allow_low_precision(reason="fp8 attn out"):
                                nc.vector.tensor_tensor(ytmp[:], yf[:], rb[:], ALU.mult)
                            nc.gpsimd.dma_start(
                                out=yT[ph:ph + 64, hp, g * 512:(g + 1) * 512],
                                in_=ytmp[:])

            mask_pool.release()
            kT_pool.release()
            qT_pool.release()

            # ---- Stage 4: proj (fp8 DR) + residual -> x1 (SBUF); LN2 -> h2T8 + lo
            x1_pool = tc.alloc_tile_pool(name="x1_pool", bufs=1)
            x1s = x1_pool.tile([128, 8, C], F32)
            h2T_pool = tc.alloc_tile_pool(name="h2T_pool", bufs=1, side="right")
            h2T = h2T_pool.tile([128, 8, R], F8)
            h2lo_pool = tc.alloc_tile_pool(name="h2lo_pool", bufs=1, side="right")
            h2loT = h2lo_pool.tile([128, 8, R], F8)
            with (
                tc.tile_pool(name="s4wp", bufs=1) as s4wp,
                tc.tile_pool(name="s4w", bufs=2) as s4w,
                tc.tile_pool(name="s4pj", bufs=2) as s4pj,
                tc.tile_pool(name="s4s", bufs=3) as s4s,
                tc.tile_pool(name="s4p", bufs=3, space="PSUM") as s4p,
                tc.tile_pool(name="s4tp", bufs=2, space="PSUM") as s4tp,
            ):
                pjw = []
                for s in range(4):
                    w4 = s4wp.tile([128, 8, 256], F8, tag=f"wpj{s}")
                    nc.sync.dma_start(out=w4[:], in_=wproj_in[s])
                    pjw.append(w4)
                for n in range(2):
                    pjt = []
                    for s in range(4):
                        for sub in range(2):
                            m = 2 * s + sub
                            acc = s4p.tile([128, 512], F32, tag="pjacc")
                            for c in range(4):
                                nc.tensor.matmul(
                                    acc[:],
                                    pjw[s][:, 2 * c:2 * c + 2, sub * 128:(sub + 1) * 128],
                                    yT[:, 2 * c:2 * c + 2, n * 512:(n + 1) * 512],
                                    start=(c == 0), stop=(c == 3), perf_mode=DRm)
                            ev = s4pj.tile([128, 512], F32, tag=f"pjev{m}")
                            nc.scalar.activation(ev[:], acc[:], AF.Identity,
                                                 bias=bproj_sb[:, m:m + 1],
                                                 scale=pe_scale)
                            pjt.append(ev)
                    for jj in range(4):
                        j = n * 4 + jj
                        xo = s4w.tile([128, C], F32, tag="xo")
                        nc.sync.dma_start(out=xo[:], in_=x_r[j])
                        for m in range(8):
                            pt = s4tp.tile([128, 128], F32, tag="pjt")
                            nc.tensor.transpose(
                                pt[:], pjt[m][:, jj * 128:(jj + 1) * 128], identity[:])
                            nc.vector.tensor_tensor(
                                x1s[:, j, m * 128:(m + 1) * 128],
                                pt[:], xo[:, m * 128:(m + 1) * 128], ALU.add)
                        h2 = s4w.tile([128, C], BF16, tag="h2")
                        _layernorm_tiles(nc, s4s, x1s[:, j, :], h2[:], "s4", eps_t[:])
                        for ci in range(8):
                            pt = s4tp.tile([128, 128], BF16, tag="h2t")
                            nc.tensor.transpose(pt[:], h2[:, ci * 128:(ci + 1) * 128],
                                                identb[:])
                            nc.vector.tensor_scalar_mul(
                                h2T[:, ci, j * 128:(j + 1) * 128], pt[:], SH2)
                            nc.vector.scalar_tensor_tensor(
                                h2loT[:, ci, j * 128:(j + 1) * 128], pt[:], SH2,
                                h2T[:, ci, j * 128:(j + 1) * 128],
                                op0=ALU.mult, op1=ALU.subtract)


            # ---- Stage 5: MLP fc1 (fp8 DR, 3-term compensated) -> gelu -> gT bf16
            gT_pool = tc.alloc_tile_pool(name="gT_pool", bufs=1)
            gT = gT_pool.tile([128, 32, R], F8)
            with (
                tc.tile_pool(name="s5w", bufs=4) as s5w,
                tc.tile_pool(name="s5p", bufs=4, space="PSUM") as s5p,
            ):
                for s in range(16):
                    wsl = s5w.tile([128, 8, 2, 256], F8, tag="wfc")
                    nc.sync.dma_start(out=wsl[:], in_=wfc_in[s])
                    for sub in range(2):
                        m = 2 * s + sub
                        for n in range(2):
                            acc = s5p.tile([128, 512], F32, tag="facc")
                            for c in range(8):
                                nc.tensor.matmul(
                                    acc[:],
                                    wsl[:, c, :, sub * 128:(sub + 1) * 128],
                                    h2T[:, c, n * 512:(n + 1) * 512]
                                    .unsqueeze(1).broadcast_to([128, 2, 512]),
                                    start=(c == 0), stop=False, perf_mode=DRm)
                            for c in range(4):
                                nc.tensor.matmul(
                                    acc[:],
                                    wsl[:, 2 * c:2 * c + 2, 0, sub * 128:(sub + 1) * 128],
                                    h2loT[:, 2 * c:2 * c + 2, n * 512:(n + 1) * 512],
                                    start=False, stop=(c == 3), perf_mode=DRm)
                            nc.scalar.activation(
                                gT[:, m, n * 512:(n + 1) * 512], acc[:],
                                AF.Gelu_apprx_tanh, bias=bfc_sb[:, m:m + 1],
                                scale=ge_scale)
            h2lo_pool.release()
            h2T_pool.release()

            # ---- Stage 6: fc2 (bf16) + residual -> out
            with (
                tc.tile_pool(name="s6w", bufs=3) as s6w,
                tc.tile_pool(name="s6ev", bufs=1) as s6ev,
                tc.tile_pool(name="s6p", bufs=3, space="PSUM") as s6p,
                tc.tile_pool(name="s6tp", bufs=2, space="PSUM") as s6tp,
            ):
                for half in range(2):
                    mev = []
                    for m2 in range(4):
                        m = half * 4 + m2
                        wsl = s6w.tile([128, 32, 2, 128], F8, tag="wfc2")
                        nc.sync.dma_start(out=wsl[:], in_=wfc2_in[m])
                        acc = s6p.tile([128, 1024], F32, tag="macc")
                        for n in range(2):
                            for df in range(32):
                                nc.tensor.matmul(
                                    acc[:, n * 512:(n + 1) * 512],
                                    wsl[:, df, :, :],
                                    gT[:, df, n * 512:(n + 1) * 512]
                                    .unsqueeze(1).broadcast_to([128, 2, 512]),
                                    start=(df == 0), stop=(df == 31), perf_mode=DRm)
                        ev = s6ev.tile([128, 1024], F32, tag=f"mev{m2}")
                        nc.scalar.activation(ev[:], acc[:], AF.Identity,
                                             bias=bfc2_sb[:, m:m + 1],
                                             scale=f2_scale)
                        mev.append(ev)
                    for j in range(8):
                        stg = s6w.tile([128, 512], F32, tag="stg")
                        for m2 in range(4):
                            pt = s6tp.tile([128, 128], F32, tag="mt")
                            nc.tensor.transpose(
                                pt[:], mev[m2][:, j * 128:(j + 1) * 128], identity[:])
                            nc.vector.tensor_tensor(
                                stg[:, m2 * 128:(m2 + 1) * 128], pt[:],
                                x1s[:, j, half * 512 + m2 * 128:
                                    half * 512 + (m2 + 1) * 128], ALU.add)
                        nc.scalar.dma_start(
                            out=out_d[j * 128:(j + 1) * 128,
                                      half * 512:(half + 1) * 512],
                            in_=stg[:])
            gT_pool.release()

    nc.compile()
    return nc


_NC = None
_NC_SCALES = None


def _host_prepare(x, ln1_w, ln1_b, w_attn, b_attn, w_proj, b_proj,
                  ln2_w, ln2_b, w_fc, b_fc, w_fc2, b_fc2):
    f32 = np.float32
    ln1_w = np.asarray(ln1_w, f32); ln1_b = np.asarray(ln1_b, f32)
    w_attn = np.asarray(w_attn, f32); b_attn = np.asarray(b_attn, f32)
    scale = np.zeros((3 * C,), f32)
    scale[:C] = 0.125
    scale[C:] = 1.0
    w_full = ln1_w[:, None] * w_attn * scale[None, :]
    b_full = (ln1_b @ w_attn + b_attn) * scale
    wqk = w_full[:, :2 * C]
    swqk = _pow2_scale(wqk)
    wv = w_full[:, 2 * C:]
    swv = _pow2_scale(wv)
    wproj = np.asarray(w_proj, f32)
    swp = _pow2_scale(wproj)
    wfc = np.asarray(ln2_w, f32)[:, None] * np.asarray(w_fc, f32)
    sw1 = _pow2_scale(wfc)
    wfc2 = np.asarray(w_fc2, f32)
    sw2 = _pow2_scale(wfc2)
    w2hi = _slab8(wfc2, sw2, 128)                    # [32, 128, 32, 128]
    w2hi_full = w2hi.astype(f32).transpose(2, 1, 0, 3).reshape(DFF, C)
    w2lo = _slab8(wfc2 - w2hi_full / sw2, sw2, 128)
    wfc2_slab = np.ascontiguousarray(np.stack([w2hi, w2lo], axis=3))
    # fc1 hi+lo fp8 split sharing one scale: slab [16, 128, 8, 2, 256]
    w1hi = _slab8(wfc, sw1, 256)                     # [16, 128, 8, 256]
    w1hi_full = w1hi.astype(f32).transpose(2, 1, 0, 3).reshape(C, DFF)
    w1lo = _slab8(wfc - w1hi_full / sw1, sw1, 256)
    wfc_slab = np.ascontiguousarray(np.stack([w1hi, w1lo], axis=3))
    # wv fp8 [128, 8, C]: [cin_part, cin_chunk, cout]
    wv8 = _q8(wv, swv).reshape(8, 128, C).transpose(1, 0, 2)
    shared = {
        "wqk": _slab8(wqk, swqk, 256),
        "bqk": np.ascontiguousarray(b_full[:2 * C]),
        "wv": np.ascontiguousarray(wv8),
        "bv": _f32r_round(b_full[2 * C:] * SH * swv),
        "wproj": _slab8(wproj, swp, 256),
        "bproj": np.asarray(b_proj, f32),
        "wfc": wfc_slab,
        "bfc": np.asarray(ln2_b, f32) @ np.asarray(w_fc, f32) + np.asarray(b_fc, f32),
        "wfc2": wfc2_slab,
        "bfc2": np.asarray(b_fc2, f32),
    }
    # masks[p]: [128 (ki within chunk), 2 (qgroup), 8 (mask slot), 512 (qi)]
    masks = []
    ki = np.arange(128)
    for p in range(2):
        mk = np.zeros((128, 2, 8, 512), f32)
        for g in range(2):
            qc_i = np.arange(512) // 128          # i within group
            qr = np.arange(512) % 128
            q_real = (2 * (4 * g + qc_i) + p) * 128 + qr    # [512]
            for slot in range(8):
                if slot < 4:
                    real_chunk = 2 * (4 * g + slot) + p      # own keys
                else:
                    real_chunk = 2 * (4 * g + slot - 4) + 1 - p  # other-parity keys
                k_real = real_chunk * 128 + ki               # [128]
                mk[:, g, slot, :] = (k_real[:, None] <= q_real[None, :])
        masks.append(mk.astype(E4))
    return shared, masks, (swqk, swv, swp, sw1, sw2)


def kernel(**inputs):
    global _NC, _NC_SCALES
    x = np.asarray(inputs["x"], np.float32)
    shared, masks, sw_scales = _host_prepare(**inputs)
    if _NC is None or _NC_SCALES != sw_scales:
        _NC = build_program(sw_scales)
        _NC_SCALES = sw_scales
    nc = _NC
    in_maps = []
    for c in range(8):
        b, p = c // 2, c % 2
        perm = [2 * j + p for j in range(8)] + [2 * j + 1 - p for j in range(8)]
        xp = np.ascontiguousarray(
            x[b].reshape(NCH, 128, C)[perm].reshape(T, C))
        im = dict(shared)
        im["x"] = xp
        im["masks"] = masks[p]
        in_maps.append(im)
    res = run_bass_kernel_spmd(nc, in_maps, list(range(8)), trace=False).results
    out = np.empty((B, T, C), np.float32)
    for c in range(8):
        b, p = c // 2, c % 2
        oc = res[c]["out"].reshape(NOWN, 128, C)
        for j in range(NOWN):
            out[b, (2 * j + p) * 128:(2 * j + p + 1) * 128, :] = oc[j]
    return out
